# revision 1
# baseline (speedup 1.0000x reference)
"""Trainium2 kernel for CrossSiloAggregator (gnn_message_passing).

Reference semantics:
    local_emb = local_embeddings[local_indices]            # [M, D] gather
    w = sigmoid(concat([local_emb, foreign], -1) @ W + b)  # [M, 1]
    updated = w * local_emb + (1 - w) * foreign            # [M, D]
    out = local_embeddings.at[local_indices].set(updated)

Strategy (8 NeuronCores, memory-bound):
  - Host gathers the M=200k boundary rows (general in local_indices),
    shards them evenly across 8 cores (25k rows each) and passes each
    shard TRANSPOSED ([D=128 partitions, rows free]).  The transposed
    layout lets the TensorEngine compute the attention logits as two
    K=128 matmuls (Wl.T @ lT + Wf.T @ fT).
  - Engine balance (measured ~118-130us/core vs ~116us DMA floor):
      PE     logits only (fp32 matmul is 4 cyc/row)           ~84us
      ACT    sigmoid per 512-slice                            ~25us
      GPSIMD partition_broadcast of w, once per chunk         cheap
      DVE    chunk-wide sub (l-f), mul (*w), add (+f)
      (GPSIMD elementwise mul measured 2x slower than DVE on real
       HW despite the cost model preferring it — keep blend on DVE)
  - Device computes only the 200k updated rows; the untouched 800k rows
    are carried to the output by the host-side unshard (a copy the
    full-IO contract requires anyway).
"""

import sys

import numpy as np

if "/opt/trn_rl_repo" not in sys.path:  # harness may run without PYTHONPATH
    sys.path.append("/opt/trn_rl_repo")

P = 128          # partitions == embedding dim
N_CORES = 8
N_FOREIGN = 200_000
ROWS_PER_CORE = N_FOREIGN // N_CORES   # 25000
CHUNK = 4096     # rows per SBUF tile
SLICE = 512      # matmul free-dim (one PSUM bank)


def _chunks(rows, chunk):
    out = []
    off = 0
    while off < rows:
        n = min(chunk, rows - off)
        out.append((off, n))
        off += n
    return out


def build_nc(rows=ROWS_PER_CORE, chunk=CHUNK, slice_n=SLICE, repeats=1,
             bufs_io=3, bufs_o=3, bufs_w=1, bufs_wb=2, bufs_log=3,
             mul_eng="dve", add_eng="dve", sub_eng="dve", skip=(),
             logit_dtype="f32", split_out=False):
    """Build the per-core Bass program (SPMD: identical on all cores).

    repeats>1 re-runs the whole pass over the same DRAM buffers (used by
    the timing harness to difference out fixed dispatch overhead)."""
    from contextlib import ExitStack

    import concourse.bacc as bacc
    import concourse.mybir as mybir
    import concourse.tile as tile

    f32 = mybir.dt.float32
    fio = mybir.dt.float32r if logit_dtype == "f32r" else f32
    nc = bacc.Bacc("TRN2")

    lT = nc.dram_tensor("lT", [P, rows], fio, kind="ExternalInput")
    fT = nc.dram_tensor("fT", [P, rows], fio, kind="ExternalInput")
    wl = nc.dram_tensor("wl", [P, 1], fio, kind="ExternalInput")
    wf = nc.dram_tensor("wf", [P, 1], fio, kind="ExternalInput")
    bb = nc.dram_tensor("bb", [1, 1], f32, kind="ExternalInput")
    outT = nc.dram_tensor("outT", [P, rows], f32, kind="ExternalOutput")

    def eng(name):
        return {"dve": nc.vector, "gpsimd": nc.gpsimd}[name]

    with tile.TileContext(nc) as tc, ExitStack() as ctx:
        consts = ctx.enter_context(tc.tile_pool(name="consts", bufs=1))
        io_l = ctx.enter_context(tc.tile_pool(name="io_l", bufs=bufs_io))
        io_f = ctx.enter_context(tc.tile_pool(name="io_f", bufs=bufs_io))
        io_o = ctx.enter_context(tc.tile_pool(name="io_o", bufs=bufs_o))
        wpool = ctx.enter_context(tc.tile_pool(name="wpool", bufs=bufs_w))
        wbpool = ctx.enter_context(tc.tile_pool(name="wbpool", bufs=bufs_wb))
        ps_log = ctx.enter_context(
            tc.tile_pool(name="ps_log", bufs=bufs_log, space="PSUM"))

        wl_sb = consts.tile([P, 1], fio)
        nc.sync.dma_start(out=wl_sb, in_=wl[:])
        wf_sb = consts.tile([P, 1], fio)
        nc.sync.dma_start(out=wf_sb, in_=wf[:])
        b_sb = consts.tile([1, 1], f32)
        nc.sync.dma_start(out=b_sb, in_=bb[:])

        for off, n in _chunks(rows, chunk) * repeats:
            nsl = (n + slice_n - 1) // slice_n

            l_t = io_l.tile([P, n], fio, tag="l")
            f_t = io_f.tile([P, n], fio, tag="f")
            o_t = io_o.tile([P, n], f32, tag="o")
            w_sb = wpool.tile([1, n], f32, tag="w")
            wb_t = wbpool.tile([P, n], f32, tag="wb")
            nc.sync.dma_start(out=l_t, in_=lT[:, off : off + n])
            nc.sync.dma_start(out=f_t, in_=fT[:, off : off + n])

            # o = l - f (chunk-wide)
            if "sub" not in skip:
                eng(sub_eng).tensor_sub(out=o_t, in0=l_t, in1=f_t)

            for s in range(nsl):
                if "logit" in skip:
                    break
                a = s * slice_n
                m = min(slice_n, n - a)
                # logits for this slice: Wl.T @ l + Wf.T @ f  (PSUM accum)
                lg = ps_log.tile([1, slice_n], f32, tag="logit")
                nc.tensor.matmul(
                    out=lg[:, :m],
                    lhsT=wl_sb[:],
                    rhs=l_t[:, a : a + m],
                    start=True,
                    stop=False,
                )
                nc.tensor.matmul(
                    out=lg[:, :m],
                    lhsT=wf_sb[:],
                    rhs=f_t[:, a : a + m],
                    start=False,
                    stop=True,
                )
                # w = sigmoid(logit + b) on ACT; sole reader of lg
                nc.scalar.activation(
                    out=w_sb[:, a : a + m],
                    in_=lg[:, :m],
                    func=mybir.ActivationFunctionType.Sigmoid,
                    bias=b_sb,
                    scale=1.0,
                )

            # broadcast w across partitions (GPSIMD), then o *= w
            if "bcast" not in skip:
                nc.gpsimd.partition_broadcast(wb_t[:, :n], w_sb[:, :n])
            if split_out and n % 2 == 0:
                # finish and store each half independently so the output DMA
                # of the first half overlaps the second half's blend tail
                h = n // 2
                for c0 in (0, h):
                    if "mul" not in skip:
                        eng(mul_eng).tensor_mul(
                            out=o_t[:, c0 : c0 + h],
                            in0=o_t[:, c0 : c0 + h],
                            in1=wb_t[:, c0 : c0 + h],
                        )
                    if "add" not in skip:
                        eng(add_eng).tensor_add(
                            out=o_t[:, c0 : c0 + h],
                            in0=o_t[:, c0 : c0 + h],
                            in1=f_t[:, c0 : c0 + h],
                        )
                    nc.sync.dma_start(
                        out=outT[:, off + c0 : off + c0 + h],
                        in_=o_t[:, c0 : c0 + h],
                    )
            else:
                if "mul" not in skip:
                    eng(mul_eng).tensor_mul(out=o_t, in0=o_t, in1=wb_t)
                # o += f
                if "add" not in skip:
                    eng(add_eng).tensor_add(out=o_t, in0=o_t, in1=f_t)

                nc.sync.dma_start(out=outT[:, off : off + n], in_=o_t)

    nc.finalize()
    return nc


_NC_CACHE = {}


def _get_nc():
    key = "main"
    if key not in _NC_CACHE:
        _NC_CACHE[key] = build_nc()
    return _NC_CACHE[key]


def make_in_maps(local_embeddings, foreign_embeddings, local_indices, W_att, b_att):
    l_rows = np.ascontiguousarray(local_embeddings[local_indices])  # [M, D]
    wl = np.ascontiguousarray(W_att[:P].reshape(P, 1), dtype=np.float32)
    wf = np.ascontiguousarray(W_att[P:].reshape(P, 1), dtype=np.float32)
    bbv = np.ascontiguousarray(np.reshape(b_att, (1, 1)), dtype=np.float32)
    in_maps = []
    for i in range(N_CORES):
        sl = slice(i * ROWS_PER_CORE, (i + 1) * ROWS_PER_CORE)
        in_maps.append(
            {
                "lT": np.ascontiguousarray(l_rows[sl].T),
                "fT": np.ascontiguousarray(foreign_embeddings[sl].T),
                "wl": wl,
                "wf": wf,
                "bb": bbv,
            }
        )
    return in_maps


def run_device(in_maps, trace=False):
    from concourse.bass_utils import run_bass_kernel_spmd

    return run_bass_kernel_spmd(
        _get_nc(), in_maps, core_ids=list(range(N_CORES)), trace=trace
    )


def kernel(local_embeddings, foreign_embeddings, local_indices, W_att, b_att):
    local_embeddings = np.asarray(local_embeddings, dtype=np.float32)
    foreign_embeddings = np.asarray(foreign_embeddings, dtype=np.float32)
    local_indices = np.asarray(local_indices)
    W_att = np.asarray(W_att, dtype=np.float32)
    b_att = np.asarray(b_att, dtype=np.float32)

    in_maps = make_in_maps(
        local_embeddings, foreign_embeddings, local_indices, W_att, b_att
    )
    res = run_device(in_maps)

    updated = np.empty((N_FOREIGN, P), dtype=np.float32)
    for i in range(N_CORES):
        sl = slice(i * ROWS_PER_CORE, (i + 1) * ROWS_PER_CORE)
        updated[sl] = res.results[i]["outT"].T

    out = local_embeddings.copy()
    out[local_indices] = updated
    return out



# revision 3
# speedup vs baseline: 1.3370x; 1.3370x over previous
"""Trainium2 kernel for CrossSiloAggregator (gnn_message_passing).

Reference semantics:
    local_emb = local_embeddings[local_indices]            # [M, D] gather
    w = sigmoid(concat([local_emb, foreign], -1) @ W + b)  # [M, 1]
    updated = w * local_emb + (1 - w) * foreign            # [M, D]
    out = local_embeddings.at[local_indices].set(updated)

Strategy (8 NeuronCores, memory-bound):
  - Host gathers the M=200k boundary rows (general in local_indices),
    shards them evenly across 8 cores (25k rows each) and passes each
    shard TRANSPOSED ([D=128 partitions, rows free]) in fp16.  The
    transposed layout lets the TensorEngine compute the attention
    logits as two K=128 matmuls (Wl.T @ lT + Wf.T @ fT).
  - fp16 end-to-end on device halves DMA traffic vs f32 (the kernel is
    DMA-bound: ~420 GB/s/core effective), runs the PE at 1 cyc/col
    (vs 4 for f32) and doubles DVE throughput.  Worst-case blend error
    ~5e-3 rel, well inside the 2e-2 gate (bf16 would be borderline).
  - Engine balance per core (25k rows):
      DMA    3 x 6.4 MB (lT, fT in; outT out)              ~46us floor
      PE     logits (2 matmuls / 512-slice, fp16)          ~21us
      ACT    sigmoid per 512-slice                         ~25us
      GPSIMD partition_broadcast of w, once per chunk      cheap
      DVE    chunk-wide sub (l-f), mul (*w), add (+f)      ~16us
  - Device computes only the 200k updated rows; the untouched 800k rows
    are carried to the output by the host-side unshard (a copy the
    full-IO contract requires anyway).
"""

import sys

import numpy as np

if "/opt/trn_rl_repo" not in sys.path:  # harness may run without PYTHONPATH
    sys.path.append("/opt/trn_rl_repo")

P = 128          # partitions == embedding dim
N_CORES = 8
N_FOREIGN = 200_000
ROWS_PER_CORE = N_FOREIGN // N_CORES   # 25000
CHUNK = 8192     # rows per SBUF tile (2 MB fp16 DMA per tile)
SLICE = 512      # matmul free-dim (one PSUM bank)


def _chunks(rows, chunk):
    out = []
    off = 0
    while off < rows:
        n = min(chunk, rows - off)
        out.append((off, n))
        off += n
    return out


def build_nc(rows=ROWS_PER_CORE, chunk=CHUNK, slice_n=SLICE, repeats=1,
             bufs_io=3, bufs_o=2, bufs_w=2, bufs_wb=2, bufs_log=3,
             mul_eng="dve", add_eng="dve", sub_eng="dve", skip=()):
    """Build the per-core Bass program (SPMD: identical on all cores).

    repeats>1 re-runs the whole pass over the same DRAM buffers (used by
    the timing harness to difference out fixed dispatch overhead)."""
    from contextlib import ExitStack

    import concourse.bacc as bacc
    import concourse.mybir as mybir
    import concourse.tile as tile

    f32 = mybir.dt.float32
    f16 = mybir.dt.float16
    nc = bacc.Bacc("TRN2")

    lT = nc.dram_tensor("lT", [P, rows], f16, kind="ExternalInput")
    fT = nc.dram_tensor("fT", [P, rows], f16, kind="ExternalInput")
    wl = nc.dram_tensor("wl", [P, 1], f16, kind="ExternalInput")
    wf = nc.dram_tensor("wf", [P, 1], f16, kind="ExternalInput")
    bb = nc.dram_tensor("bb", [1, 1], f32, kind="ExternalInput")
    outT = nc.dram_tensor("outT", [P, rows], f16, kind="ExternalOutput")

    def eng(name):
        return {"dve": nc.vector, "gpsimd": nc.gpsimd}[name]

    with tile.TileContext(nc) as tc, ExitStack() as ctx:
        consts = ctx.enter_context(tc.tile_pool(name="consts", bufs=1))
        io_l = ctx.enter_context(tc.tile_pool(name="io_l", bufs=bufs_io))
        io_f = ctx.enter_context(tc.tile_pool(name="io_f", bufs=bufs_io))
        io_o = ctx.enter_context(tc.tile_pool(name="io_o", bufs=bufs_o))
        wpool = ctx.enter_context(tc.tile_pool(name="wpool", bufs=bufs_w))
        wbpool = ctx.enter_context(tc.tile_pool(name="wbpool", bufs=bufs_wb))
        ps_log = ctx.enter_context(
            tc.tile_pool(name="ps_log", bufs=bufs_log, space="PSUM"))

        wl_sb = consts.tile([P, 1], f16)
        nc.sync.dma_start(out=wl_sb, in_=wl[:])
        wf_sb = consts.tile([P, 1], f16)
        nc.sync.dma_start(out=wf_sb, in_=wf[:])
        b_sb = consts.tile([1, 1], f32)
        nc.sync.dma_start(out=b_sb, in_=bb[:])

        for off, n in _chunks(rows, chunk) * repeats:
            nsl = (n + slice_n - 1) // slice_n

            l_t = io_l.tile([P, n], f16, tag="l")
            f_t = io_f.tile([P, n], f16, tag="f")
            o_t = io_o.tile([P, n], f16, tag="o")
            w_sb = wpool.tile([1, n], f16, tag="w")
            wb_t = wbpool.tile([P, n], f16, tag="wb")
            nc.sync.dma_start(out=l_t, in_=lT[:, off : off + n])
            nc.sync.dma_start(out=f_t, in_=fT[:, off : off + n])

            # o = l - f (chunk-wide)
            if "sub" not in skip:
                eng(sub_eng).tensor_sub(out=o_t, in0=l_t, in1=f_t)

            for s in range(nsl):
                if "logit" in skip:
                    break
                a = s * slice_n
                m = min(slice_n, n - a)
                # logits for this slice: Wl.T @ l + Wf.T @ f  (PSUM accum)
                lg = ps_log.tile([1, slice_n], f32, tag="logit")
                nc.tensor.matmul(
                    out=lg[:, :m],
                    lhsT=wl_sb[:],
                    rhs=l_t[:, a : a + m],
                    start=True,
                    stop=False,
                )
                nc.tensor.matmul(
                    out=lg[:, :m],
                    lhsT=wf_sb[:],
                    rhs=f_t[:, a : a + m],
                    start=False,
                    stop=True,
                )
                # w = sigmoid(logit + b) on ACT; sole reader of lg
                nc.scalar.activation(
                    out=w_sb[:, a : a + m],
                    in_=lg[:, :m],
                    func=mybir.ActivationFunctionType.Sigmoid,
                    bias=b_sb,
                    scale=1.0,
                )

            # broadcast w across partitions (GPSIMD), then o *= w, o += f
            if "bcast" not in skip:
                nc.gpsimd.partition_broadcast(wb_t[:, :n], w_sb[:, :n])
            if "mul" not in skip:
                eng(mul_eng).tensor_mul(out=o_t, in0=o_t, in1=wb_t)
            if "add" not in skip:
                eng(add_eng).tensor_add(out=o_t, in0=o_t, in1=f_t)

            nc.sync.dma_start(out=outT[:, off : off + n], in_=o_t)

    nc.finalize()
    return nc


_NC_CACHE = {}


def _get_nc():
    key = "main"
    if key not in _NC_CACHE:
        _NC_CACHE[key] = build_nc()
    return _NC_CACHE[key]


def make_in_maps(local_embeddings, foreign_embeddings, local_indices, W_att, b_att):
    l_rows = local_embeddings[local_indices]  # [M, D] host gather
    wl = np.ascontiguousarray(W_att[:P].reshape(P, 1), dtype=np.float16)
    wf = np.ascontiguousarray(W_att[P:].reshape(P, 1), dtype=np.float16)
    bbv = np.ascontiguousarray(np.reshape(b_att, (1, 1)), dtype=np.float32)
    in_maps = []
    for i in range(N_CORES):
        sl = slice(i * ROWS_PER_CORE, (i + 1) * ROWS_PER_CORE)
        in_maps.append(
            {
                "lT": np.ascontiguousarray(l_rows[sl].T, dtype=np.float16),
                "fT": np.ascontiguousarray(foreign_embeddings[sl].T, dtype=np.float16),
                "wl": wl,
                "wf": wf,
                "bb": bbv,
            }
        )
    return in_maps


def run_device(in_maps, trace=False):
    from concourse.bass_utils import run_bass_kernel_spmd

    return run_bass_kernel_spmd(
        _get_nc(), in_maps, core_ids=list(range(N_CORES)), trace=trace
    )


def kernel(local_embeddings, foreign_embeddings, local_indices, W_att, b_att):
    local_embeddings = np.asarray(local_embeddings, dtype=np.float32)
    foreign_embeddings = np.asarray(foreign_embeddings, dtype=np.float32)
    local_indices = np.asarray(local_indices)
    W_att = np.asarray(W_att, dtype=np.float32)
    b_att = np.asarray(b_att, dtype=np.float32)

    in_maps = make_in_maps(
        local_embeddings, foreign_embeddings, local_indices, W_att, b_att
    )
    res = run_device(in_maps)

    updated = np.empty((N_FOREIGN, P), dtype=np.float32)
    for i in range(N_CORES):
        sl = slice(i * ROWS_PER_CORE, (i + 1) * ROWS_PER_CORE)
        updated[sl] = res.results[i]["outT"].T.astype(np.float32)

    out = local_embeddings.copy()
    out[local_indices] = updated
    return out


# revision 11
# speedup vs baseline: 1.7625x; 1.3183x over previous
"""Trainium2 kernel for CrossSiloAggregator (gnn_message_passing).

Reference semantics:
    local_emb = local_embeddings[local_indices]            # [M, D] gather
    w = sigmoid(concat([local_emb, foreign], -1) @ W + b)  # [M, 1]
    updated = w * local_emb + (1 - w) * foreign            # [M, D]
    out = local_embeddings.at[local_indices].set(updated)

Strategy (8 NeuronCores, memory-bound):
  - Host gathers the M=200k boundary rows (general in local_indices),
    shards them evenly across 8 cores (25k rows each) and passes each
    shard TRANSPOSED ([D=128 partitions, rows free]) in fp16.  fp16
    end-to-end halves DMA traffic vs f32 (the kernel is DMA-bound at
    ~430 GB/s/core) and keeps worst-case blend error ~5e-3 rel, inside
    the 2e-2 gate (bf16 would be borderline).
  - The attention weight vectors are passed COLUMN-REPLICATED
    ([128, 128] with every column = wl): the logit matmul then writes
    identical logits into all 128 PSUM partitions, so the sigmoid on
    ACT lands an already-broadcast w tile in SBUF.  This deletes the
    GPSIMD partition_broadcast (which serialized against DVE via the
    shared SBUF port, +28us on the critical path) at zero PE/ACT cost
    (matmul time is N-bound; ACT lanes are per-partition parallel).
  - Engine balance per core (25k rows):
      DMA    3 x 6.4 MB (lT, fT in; outT out)              ~45us floor
      PE     logits (2 matmuls / 512-slice, fp16)          ~26us
      ACT    sigmoid per 512-slice -> broadcast w tile     ~26us
      DVE    chunk-wide sub (l-f), mul (*w), add (+f)      ~27us
  - Device computes only the 200k updated rows; the untouched 800k rows
    are carried to the output by the host-side unshard (a copy the
    full-IO contract requires anyway).
"""

import sys

import numpy as np

if "/opt/trn_rl_repo" not in sys.path:  # harness may run without PYTHONPATH
    sys.path.append("/opt/trn_rl_repo")

P = 128          # partitions == embedding dim
N_CORES = 8
N_FOREIGN = 200_000
ROWS_PER_CORE = N_FOREIGN // N_CORES   # 25000
CHUNK = 8192     # rows per SBUF tile (2 MB fp16 DMA per tile)
SLICE = 512      # matmul free-dim (one PSUM bank)


def _chunks(rows, chunk):
    out = []
    off = 0
    while off < rows:
        n = min(chunk, rows - off)
        out.append((off, n))
        off += n
    return out


def build_nc(rows=ROWS_PER_CORE, chunk=CHUNK, slice_n=SLICE, repeats=1,
             bufs_io=3, bufs_o=2, bufs_wb=2, bufs_log=3,
             mul_eng="dve", add_eng="dve", sub_eng="dve", skip=(),
             store_from_f=False, slice_mul=False):
    """Build the per-core Bass program (SPMD: identical on all cores).

    repeats>1 re-runs the whole pass over the same DRAM buffers (used by
    the timing harness to difference out fixed dispatch overhead)."""
    from contextlib import ExitStack

    import concourse.bacc as bacc
    import concourse.mybir as mybir
    import concourse.tile as tile

    f32 = mybir.dt.float32
    f16 = mybir.dt.float16
    nc = bacc.Bacc("TRN2")

    lT = nc.dram_tensor("lT", [P, rows], f16, kind="ExternalInput")
    fT = nc.dram_tensor("fT", [P, rows], f16, kind="ExternalInput")
    # attention weight vectors, column-replicated to [P, P] on the host
    wl = nc.dram_tensor("wl", [P, P], f16, kind="ExternalInput")
    wf = nc.dram_tensor("wf", [P, P], f16, kind="ExternalInput")
    # bias replicated to [P, 1] on the host
    bb = nc.dram_tensor("bb", [P, 1], f32, kind="ExternalInput")
    outT = nc.dram_tensor("outT", [P, rows], f16, kind="ExternalOutput")

    def eng(name):
        return {"dve": nc.vector, "gpsimd": nc.gpsimd}[name]

    with tile.TileContext(nc) as tc, ExitStack() as ctx:
        consts = ctx.enter_context(tc.tile_pool(name="consts", bufs=1))
        io_l = ctx.enter_context(tc.tile_pool(name="io_l", bufs=bufs_io))
        io_f = ctx.enter_context(tc.tile_pool(name="io_f", bufs=bufs_io))
        io_o = ctx.enter_context(tc.tile_pool(name="io_o", bufs=bufs_o))
        wbpool = ctx.enter_context(tc.tile_pool(name="wbpool", bufs=bufs_wb))
        ps_log = ctx.enter_context(
            tc.tile_pool(name="ps_log", bufs=bufs_log, space="PSUM"))

        wl_sb = consts.tile([P, P], f16)
        nc.sync.dma_start(out=wl_sb, in_=wl[:])
        wf_sb = consts.tile([P, P], f16)
        nc.sync.dma_start(out=wf_sb, in_=wf[:])
        b_sb = consts.tile([P, 1], f32)
        nc.sync.dma_start(out=b_sb, in_=bb[:])

        for off, n in _chunks(rows, chunk) * repeats:
            nsl = (n + slice_n - 1) // slice_n

            l_t = io_l.tile([P, n], f16, tag="l")
            f_t = io_f.tile([P, n], f16, tag="f")
            o_t = io_o.tile([P, n], f16, tag="o")
            wb_t = wbpool.tile([P, n], f16, tag="wb")
            if "load" not in skip:
                nc.sync.dma_start(out=l_t, in_=lT[:, off : off + n])
                nc.sync.dma_start(out=f_t, in_=fT[:, off : off + n])

            # o = l - f (chunk-wide)
            if "sub" not in skip:
                eng(sub_eng).tensor_sub(out=o_t, in0=l_t, in1=f_t)

            for s in range(nsl):
                if "logit" in skip:
                    break
                a = s * slice_n
                m = min(slice_n, n - a)
                # broadcast logits for this slice: every PSUM partition
                # row gets wl.l + wf.f (lhsT columns are all identical)
                lg = ps_log.tile([P, slice_n], f32, tag="logit")
                nc.tensor.matmul(
                    out=lg[:, :m],
                    lhsT=wl_sb[:],
                    rhs=l_t[:, a : a + m],
                    start=True,
                    stop=False,
                )
                nc.tensor.matmul(
                    out=lg[:, :m],
                    lhsT=wf_sb[:],
                    rhs=f_t[:, a : a + m],
                    start=False,
                    stop=True,
                )
                # wb = sigmoid(logit + b) on ACT; broadcast across all
                # partitions already, lands directly in the fp16 w tile
                nc.scalar.activation(
                    out=wb_t[:, a : a + m],
                    in_=lg[:, :m],
                    func=mybir.ActivationFunctionType.Sigmoid,
                    bias=b_sb,
                    scale=1.0,
                )
                if slice_mul and "mul" not in skip:
                    eng(mul_eng).tensor_mul(
                        out=o_t[:, a : a + m],
                        in0=o_t[:, a : a + m],
                        in1=wb_t[:, a : a + m],
                    )

            # o *= wb; o += f (chunk-wide)
            if not slice_mul and "mul" not in skip:
                eng(mul_eng).tensor_mul(out=o_t, in0=o_t, in1=wb_t)
            if "add" not in skip:
                eng(add_eng).tensor_add(out=o_t, in0=o_t, in1=f_t)

            if "store" not in skip:
                nc.sync.dma_start(
                    out=outT[:, off : off + n],
                    in_=f_t if store_from_f else o_t,
                )

    nc.finalize()
    return nc


_NC_CACHE = {}


def _get_nc():
    key = "main"
    if key not in _NC_CACHE:
        _NC_CACHE[key] = build_nc()
    return _NC_CACHE[key]


def make_in_maps(local_embeddings, foreign_embeddings, local_indices, W_att, b_att):
    l_rows = local_embeddings[local_indices]  # [M, D] host gather
    wl = np.ascontiguousarray(
        np.tile(W_att[:P].reshape(P, 1), (1, P)), dtype=np.float16)
    wf = np.ascontiguousarray(
        np.tile(W_att[P:].reshape(P, 1), (1, P)), dtype=np.float16)
    bbv = np.ascontiguousarray(
        np.full((P, 1), np.reshape(b_att, ()), dtype=np.float32))
    in_maps = []
    for i in range(N_CORES):
        sl = slice(i * ROWS_PER_CORE, (i + 1) * ROWS_PER_CORE)
        in_maps.append(
            {
                "lT": np.ascontiguousarray(l_rows[sl].T, dtype=np.float16),
                "fT": np.ascontiguousarray(foreign_embeddings[sl].T, dtype=np.float16),
                "wl": wl,
                "wf": wf,
                "bb": bbv,
            }
        )
    return in_maps


def run_device(in_maps, trace=False):
    from concourse.bass_utils import run_bass_kernel_spmd

    return run_bass_kernel_spmd(
        _get_nc(), in_maps, core_ids=list(range(N_CORES)), trace=trace
    )


def kernel(local_embeddings, foreign_embeddings, local_indices, W_att, b_att):
    local_embeddings = np.asarray(local_embeddings, dtype=np.float32)
    foreign_embeddings = np.asarray(foreign_embeddings, dtype=np.float32)
    local_indices = np.asarray(local_indices)
    W_att = np.asarray(W_att, dtype=np.float32)
    b_att = np.asarray(b_att, dtype=np.float32)

    in_maps = make_in_maps(
        local_embeddings, foreign_embeddings, local_indices, W_att, b_att
    )
    res = run_device(in_maps)

    updated = np.empty((N_FOREIGN, P), dtype=np.float32)
    for i in range(N_CORES):
        sl = slice(i * ROWS_PER_CORE, (i + 1) * ROWS_PER_CORE)
        updated[sl] = res.results[i]["outT"].T.astype(np.float32)

    out = local_embeddings.copy()
    out[local_indices] = updated
    return out
